# revision 12
# baseline (speedup 1.0000x reference)
"""Trainium2 Bass kernel for Conv2Demod (StyleGAN2-style modulated conv).

Reference computation (per sample b):
    w[b,o,i,ky,kx] = weight[o,i,ky,kx] * (1 + s[b,i])
    d[b,o]         = rsqrt(sum_{i,ky,kx} w^2 + 1e-8)
    out[b]         = conv2d(img[b], w[b]*d[b,o], pad=1)

v4: 1-D Winograd F(4,3) along y (2x fewer PE matmuls than direct conv;
576 N=512 bf16 matmuls at the measured ~270 ns/mm floor = 512-col
streaming + serialized FWL LDWEIGHTS). The body is organized in four
staggered-reset stages so that, in the timing loop, iteration n+1's
image DMA + y-transform (the ~25 us DVE critical path) overlaps
iteration n's second-half matmuls:

  stage0: s/at DMAs, 1+s scalars, img half-0 DMA+modulate+transform
  stage1: U DMAs, demod matvec, img half-1 DMA+modulate+transform,
          half-0 o-tiles 0-1 matmul groups + inverse
  stage2: half-0 o-tiles 2-3
  stage3: half-1 o-tiles 0-3

V tiles are per (i-tile, row-half) so stage-n+1 writes never collide
with stage-3 reads of the other half. Inverse transform reads the 6
Winograd planes straight from 6 PSUM banks (DVE, one PSUM operand per
op; m1/m3 staged via ACT copies), demod d[o] is applied by the final
ACT activations which also pack [ty,dy,x] so one contiguous DMA per
(o-tile, half) writes the output.

Sharding: data-parallel over batch -- 8 samples onto 8 NeuronCores.
"""

import contextlib

import numpy as np
import ml_dtypes

import concourse.bacc as bacc
import concourse.mybir as mybir
import concourse.tile as tile
from concourse.bass_utils import run_bass_kernel_spmd

P = 128          # partitions
CIN = 512
COUT = 512
H = W = 64
KS = 3
MO = 4           # output rows per Winograd tile
NA = MO + KS - 1  # 6 transform planes
NK = NA * KS     # 18 U planes
NI = CIN // P    # 4 i-tiles
NO = COUT // P   # 4 o-tiles
TY = H // MO     # 16 tile-rows
TCH = 2          # row-halves
TROWS = TY // TCH  # 8 tile-rows per half
NPIX = TROWS * W   # 512 = matmul N
IRH = MO * TROWS + 2  # 34 input rows per half
HP = H + 2
WP = W + 2
EPS = 1e-8
N_CORES = 8

F32 = mybir.dt.float32
BF16 = mybir.dt.bfloat16
AF = mybir.ActivationFunctionType
OP = mybir.AluOpType
ET = mybir.EngineType
A_ORDER = (1, 2, 3, 4, 0, 5)
_nullcm = contextlib.nullcontext


def build_nc(chain=False, loop_n=None, staggered=True):
    """Per-core program: one sample's modulated conv via 1-D F(4,3)."""
    nc = bacc.Bacc("TRN2", target_bir_lowering=False, debug=False)

    img = nc.dram_tensor("img", [NI, P, HP, WP], BF16, kind="ExternalInput").ap()
    s_in = nc.dram_tensor("s", [CIN], F32, kind="ExternalInput").ap()
    ut = nc.dram_tensor("ut", [NK, CIN, COUT], BF16, kind="ExternalInput").ap()
    at = nc.dram_tensor("at", [CIN, COUT], BF16, kind="ExternalInput").ap()
    out = nc.dram_tensor("out", [COUT, H, W], F32, kind="ExternalOutput").ap()
    s_out = None
    if chain:
        s_out = nc.dram_tensor("s_out", [CIN], F32, kind="ExternalOutput").ap()

    use_stages = bool(loop_n) and staggered

    with tile.TileContext(nc) as tc:
        with (
            tc.tile_pool(name="const", bufs=1) as cpool,
            tc.tile_pool(name="persist", bufs=1) as ppool,
            tc.tile_pool(name="imgst", bufs=2) as imgst,
            tc.tile_pool(name="w1", bufs=3) as w1pool,
            tc.tile_pool(name="inv", bufs=12) as invpool,
            tc.tile_pool(name="ypool", bufs=2) as ypool,
            tc.tile_pool(name="psum", bufs=8, space="PSUM") as psum_pool,
        ):
            loop_kw = dict(
                hint_engines=(ET.PE, ET.DVE, ET.Activation),
            )
            if use_stages:
                loop_kw["staggered_reset"] = True
            with (tc.For_i(0, loop_n, 1, **loop_kw) if loop_n else _nullcm()):
                # ================= stage 0 =================================
                sraw = cpool.tile([P, NI, 2], F32, tag="sraw")
                for c in range(2):
                    nc.sync.dma_start(
                        sraw[:, :, c], s_in.rearrange("(t p) -> p t", p=P)
                    )
                if chain:
                    nc.sync.dma_start(s_out[:], s_in[:])
                smod = cpool.tile([P, NI, 2], F32, tag="smod")  # 1 + s
                nc.scalar.activation(smod[:], sraw[:], AF.Copy, bias=1.0)
                tsq = cpool.tile([P, NI, 2], BF16, tag="tsq")   # (1 + s)^2
                nc.scalar.square(tsq[:], smod[:])
                at_sb = ppool.tile([P, NI, COUT], BF16, tag="at_sb")
                nc.sync.dma_start(at_sb[:], at.rearrange("(t p) o -> p t o", p=P))

                # per-(it, half) V tiles
                vsb = [
                    [
                        ppool.tile(
                            [P, NA, TROWS, WP], BF16,
                            name=f"vsb{it}_{tch}", tag=f"vsb{it}_{tch}",
                        )
                        for tch in range(TCH)
                    ]
                    for it in range(NI)
                ]

                def transform_half(it, tch):
                    """DMA + modulate + B^T transform of one image half."""
                    irh = imgst.tile([P, IRH, WP], BF16, name="irh", tag="irh")
                    nc.sync.dma_start(
                        irh[:], img[it][:, MO * TROWS * tch : MO * TROWS * tch + IRH, :]
                    )
                    nc.scalar.activation(
                        irh[:], irh[:], AF.Copy, scale=smod[:, it, 0:1]
                    )
                    dd = [irh[:, k : k + MO * (TROWS - 1) + 1 : MO, :] for k in range(NA)]
                    v = vsb[it][tch]

                    def w1(nm):
                        return w1pool.tile([P, TROWS, WP], BF16, name=nm, tag="w1")

                    p_ = w1("p_")
                    nc.vector.tensor_tensor(p_[:], dd[1], dd[2], OP.add)
                    q_ = w1("q_")
                    nc.vector.tensor_tensor(q_[:], dd[3], dd[4], OP.add)
                    nc.vector.scalar_tensor_tensor(
                        v[:, 1], p_[:], -4.0, q_[:], OP.mult, OP.add
                    )
                    e_ = w1("e_")
                    nc.vector.tensor_tensor(e_[:], dd[1], dd[2], OP.subtract)
                    f_ = w1("f_")
                    nc.vector.tensor_tensor(f_[:], dd[3], dd[4], OP.subtract)
                    nc.vector.scalar_tensor_tensor(
                        v[:, 2], e_[:], 4.0, f_[:], OP.mult, OP.subtract
                    )
                    g_ = w1("g_")
                    nc.vector.tensor_tensor(g_[:], dd[3], dd[1], OP.subtract)
                    v_ = w1("v_")
                    nc.vector.tensor_tensor(v_[:], dd[2], dd[4], OP.subtract)
                    # h = d4 - d2 = -v_  ->  r3 = 2g - v_, r4 = -2g - v_
                    nc.vector.scalar_tensor_tensor(
                        v[:, 3], g_[:], 2.0, v_[:], OP.mult, OP.subtract
                    )
                    nc.vector.scalar_tensor_tensor(
                        v[:, 4], g_[:], -2.0, v_[:], OP.mult, OP.subtract
                    )
                    u_ = w1("u_")
                    nc.vector.tensor_tensor(u_[:], dd[0], dd[2], OP.subtract)
                    nc.vector.scalar_tensor_tensor(
                        v[:, 0], u_[:], 4.0, v_[:], OP.mult, OP.subtract
                    )
                    z_ = w1("z_")
                    nc.vector.tensor_tensor(z_[:], dd[3], dd[5], OP.subtract)
                    nc.vector.scalar_tensor_tensor(
                        v[:, 5], g_[:], -4.0, z_[:], OP.mult, OP.subtract
                    )

                for it in range(NI):
                    transform_half(it, 0)

                if use_stages:
                    tc.stage_boundary()

                # ================= stage 1 =================================
                # transformed weights U (reloaded per iteration; chunked so
                # the first a-groups' weights land first)
                usb = [
                    ppool.tile([P, NK, COUT], BF16, name=f"usb{it}", tag=f"usb{it}")
                    for it in range(NI)
                ]
                for a in A_ORDER:
                    for it in range(NI):
                        nc.sync.dma_start(
                            usb[it][:, a * KS : (a + 1) * KS, :],
                            ut[a * KS : (a + 1) * KS, it * P : (it + 1) * P, :]
                            .rearrange("k p o -> p k o"),
                        )

                # demod d[o]
                dsb = cpool.tile([P, NO], F32, tag="dsb")
                dtmp = cpool.tile([P, NO], F32, tag="dtmp")
                epst = cpool.tile([P, 1], F32, tag="epst")
                nc.vector.memset(epst[:], EPS)
                for ot in range(NO):
                    o0 = ot * P
                    psd = psum_pool.tile([P, 2], F32, name="psd", tag="ps")
                    for it in range(NI):
                        nc.tensor.matmul(
                            psd[:],
                            at_sb[:, it, o0 : o0 + P],
                            tsq[:, it, :],
                            start=(it == 0),
                            stop=(it == NI - 1),
                        )
                    nc.scalar.activation(
                        dtmp[:, ot : ot + 1], psd[:, 0:1], AF.Sqrt, bias=epst[:]
                    )
                nc.vector.reciprocal(dsb[:], dtmp[:])

                def conv_block(ot, tch):
                    """6 matmul groups + PSUM-direct inverse + pack + DMA."""
                    o0 = ot * P
                    ps = {}
                    for a in A_ORDER:
                        p_ps = psum_pool.tile([P, NPIX], F32, name=f"ps{a}", tag="ps")
                        ps[a] = p_ps
                        j = 0
                        for it in range(NI):
                            for kx in range(KS):
                                nc.tensor.matmul(
                                    p_ps[:],
                                    usb[it][:, a * KS + kx, o0 : o0 + P],
                                    vsb[it][tch][:, a, :, kx : kx + W],
                                    start=(j == 0),
                                    stop=(j == NK - 1),
                                )
                                j += 1

                    def iv(nm):
                        return invpool.tile([P, NPIX], F32, name=nm, tag="iv")

                    e1 = iv("e1")
                    nc.scalar.activation(e1[:], ps[1][:], AF.Copy)
                    e3 = iv("e3")
                    nc.scalar.activation(e3[:], ps[3][:], AF.Copy)
                    a1 = iv("a1")
                    nc.vector.tensor_tensor(a1[:], e1[:], ps[2][:], OP.subtract)
                    c1 = iv("c1")
                    nc.vector.tensor_tensor(c1[:], e1[:], ps[2][:], OP.add)
                    b1 = iv("b1")
                    nc.vector.tensor_tensor(b1[:], e3[:], ps[4][:], OP.subtract)
                    d1 = iv("d1")
                    nc.vector.tensor_tensor(d1[:], e3[:], ps[4][:], OP.add)
                    t_ = iv("t_")
                    nc.vector.tensor_tensor(t_[:], ps[0][:], c1[:], OP.add)
                    tt = iv("tt")
                    nc.vector.tensor_tensor(tt[:], t_[:], d1[:], OP.add)
                    y1r = iv("y1r")
                    nc.vector.scalar_tensor_tensor(
                        y1r[:], b1[:], 2.0, a1[:], OP.mult, OP.add
                    )
                    y2r = iv("y2r")
                    nc.vector.scalar_tensor_tensor(
                        y2r[:], d1[:], 4.0, c1[:], OP.mult, OP.add
                    )
                    y3r = iv("y3r")
                    nc.vector.scalar_tensor_tensor(
                        y3r[:], b1[:], 8.0, a1[:], OP.mult, OP.add
                    )
                    y3 = iv("y3")
                    nc.vector.tensor_tensor(y3[:], y3r[:], ps[5][:], OP.add)

                    y = ypool.tile([P, TROWS, MO, W], F32, name="y", tag="y")
                    for dy, src_t in enumerate((tt, y1r, y2r, y3)):
                        nc.scalar.activation(
                            y[:, :, dy, :],
                            src_t[:].rearrange("p (t x) -> p t x", x=W),
                            AF.Copy,
                            scale=dsb[:, ot : ot + 1],
                        )
                    r0 = (tch * TROWS) * MO
                    nc.sync.dma_start(
                        out[o0 : o0 + P, r0 : r0 + TROWS * MO, :],
                        y[:].rearrange("p t d x -> p (t d) x"),
                    )

                # half-1 transforms interleaved with half-0 ot0/ot1 blocks
                transform_half(0, 1)
                transform_half(1, 1)
                conv_block(0, 0)
                transform_half(2, 1)
                transform_half(3, 1)
                conv_block(1, 0)

                if use_stages:
                    tc.stage_boundary()
                # ================= stage 2 =================================
                conv_block(2, 0)
                conv_block(3, 0)

                if use_stages:
                    tc.stage_boundary()
                # ================= stage 3 =================================
                for ot in range(NO):
                    conv_block(ot, 1)
    nc.compile()
    return nc


_NC_CACHE = None


def _get_nc():
    global _NC_CACHE
    if _NC_CACHE is None:
        _NC_CACHE = build_nc()
    return _NC_CACHE


def make_in_maps(img, s, weight):
    """Host-side input prep: shard over batch, static weight transforms."""
    img = np.asarray(img, dtype=np.float32)
    s = np.ascontiguousarray(np.asarray(s, dtype=np.float32))
    weight = np.asarray(weight, dtype=np.float32)
    bf = ml_dtypes.bfloat16
    imgp = np.zeros((img.shape[0], NI, P, HP, WP), dtype=bf)
    imgp[:, :, :, 1 : H + 1, 1 : W + 1] = img.reshape(-1, NI, P, H, W)
    # F(4,3) G (points 0,1,-1,2,-2,inf)
    G = np.array(
        [
            [1 / 4, 0, 0],
            [-1 / 6, -1 / 6, -1 / 6],
            [-1 / 6, 1 / 6, -1 / 6],
            [1 / 24, 1 / 12, 1 / 6],
            [1 / 24, -1 / 12, 1 / 6],
            [0, 0, 1],
        ],
        np.float64,
    )
    utv = np.einsum("ag,oigx->axio", G, weight.astype(np.float64))
    utv = np.ascontiguousarray(utv.reshape(NK, CIN, COUT)).astype(bf)
    atv = np.ascontiguousarray(
        (weight.astype(np.float64) ** 2).sum(axis=(2, 3)).T
    ).astype(bf)
    return [
        {"img": imgp[b], "s": s[b], "ut": utv, "at": atv} for b in range(N_CORES)
    ]


def kernel(img, s, weight):
    nc = _get_nc()
    in_maps = make_in_maps(img, s, weight)
    res = run_bass_kernel_spmd(nc, in_maps, list(range(N_CORES)))
    return np.stack([res.results[b]["out"] for b in range(N_CORES)], axis=0)


# revision 13
# speedup vs baseline: 1.1928x; 1.1928x over previous
"""Trainium2 Bass kernel for Conv2Demod (StyleGAN2-style modulated conv).

Reference computation (per sample b):
    w[b,o,i,ky,kx] = weight[o,i,ky,kx] * (1 + s[b,i])
    d[b,o]         = rsqrt(sum_{i,ky,kx} w^2 + 1e-8)
    out[b]         = conv2d(img[b], w[b]*d[b,o], pad=1)

v3: 1-D Winograd F(4,3) along y cuts PE work 2x vs direct conv (576 vs
1152 N=512 matmuls/sample; the per-matmul cost on this toolchain is
~270 ns = 512/2.4GHz streaming + a serialized ~56 ns FWL LDWEIGHTS, so
matmul COUNT is the roofline). bf16 operands, fp32 PSUM accumulation.

Per-sample algebra:
  - modulation (1+s[i]) folded into the image (per-partition scale, DVE),
  - demodulation d[o] applied by ACT on the final inverse-transform
    planes (the only PSUM->SBUF copy; no separate m staging -- the 6
    Winograd planes of a chunk live in 6 of the 8 PSUM banks and the
    inverse transform consumes them directly with DVE ops),
  - d[o] from the tiny matvec  A_T[i,o] @ (1+s[i])^2  (A_T host-side),
  - y-transform B^T (points 0,+-1,+-2,inf) factored into 9 tensor_tensor
    + 6 scalar_tensor_tensor DVE ops per i-tile,
  - x-direction stays a direct 3-tap conv via shifted rhs reads,
  - U[a,kx,i,o] = sum_ky G[a,ky] w[o,i,ky,kx] host-precomputed in bf16,
  - inverse transform At=[[1,1,1,1,1,0],[0,1,-1,2,-2,0],[0,1,1,4,4,0],
    [0,1,-1,8,-8,1]] factored into 6 TT + 3 STT + 1 TT ops (fp32),
    final scale+pack via 4 ACT activations into a [ty,dy,x] tile whose
    flat layout equals output row order -> one contiguous DMA per chunk.

Sharding: data-parallel over batch -- 8 samples onto 8 NeuronCores.
"""

import contextlib

import numpy as np
import ml_dtypes

import concourse.bacc as bacc
import concourse.mybir as mybir
import concourse.tile as tile
from concourse.bass_utils import run_bass_kernel_spmd

P = 128          # partitions
CIN = 512
COUT = 512
H = W = 64
KS = 3
MO = 4           # output rows per Winograd tile
NA = MO + KS - 1  # 6 transform planes
NK = NA * KS     # 18 U planes
NI = CIN // P    # 4 i-tiles
NO = COUT // P   # 4 o-tiles
TY = H // MO     # 16 tile-rows
TCH = 2          # tile-row chunks
TROWS = TY // TCH  # 8 tile-rows per chunk
NPIX = TROWS * W   # 512 = matmul N
HP = H + 2
WP = W + 2
EPS = 1e-8
N_CORES = 8

F32 = mybir.dt.float32
BF16 = mybir.dt.bfloat16
AF = mybir.ActivationFunctionType
OP = mybir.AluOpType
_nullcm = contextlib.nullcontext


def build_nc(chain=False, loop_n=None, staggered=False):
    """Per-core program: one sample's modulated conv via 1-D F(4,3)."""
    nc = bacc.Bacc("TRN2", target_bir_lowering=False, debug=False)

    img = nc.dram_tensor("img", [NI, P, HP, WP], BF16, kind="ExternalInput").ap()
    s_in = nc.dram_tensor("s", [CIN], F32, kind="ExternalInput").ap()
    ut = nc.dram_tensor("ut", [NK, CIN, COUT], BF16, kind="ExternalInput").ap()
    at = nc.dram_tensor("at", [CIN, COUT], BF16, kind="ExternalInput").ap()
    out = nc.dram_tensor("out", [COUT, H, W], F32, kind="ExternalOutput").ap()
    s_out = None
    if chain:
        s_out = nc.dram_tensor("s_out", [CIN], F32, kind="ExternalOutput").ap()

    with tile.TileContext(nc) as tc:
        with (
            tc.tile_pool(name="const", bufs=1) as cpool,
            tc.tile_pool(name="persist", bufs=1) as ppool,
            tc.tile_pool(name="imgst", bufs=2) as imgst,
            tc.tile_pool(name="w1", bufs=3) as w1pool,
            tc.tile_pool(name="inv", bufs=12) as invpool,
            tc.tile_pool(name="ypool", bufs=2) as ypool,
            tc.tile_pool(name="psum", bufs=8, space="PSUM") as psum_pool,
        ):
            loop_kw = dict(
                hint_engines=(
                    mybir.EngineType.PE,
                    mybir.EngineType.DVE,
                    mybir.EngineType.Activation,
                ),
            )
            if staggered:
                loop_kw["staggered_reset"] = True
            with (tc.For_i(0, loop_n, 1, **loop_kw) if loop_n else _nullcm()):
                # ---- s-derived scalars -------------------------------------
                sraw = cpool.tile([P, NI, 2], F32, tag="sraw")
                for c in range(2):
                    nc.sync.dma_start(
                        sraw[:, :, c], s_in.rearrange("(t p) -> p t", p=P)
                    )
                if chain:
                    nc.sync.dma_start(s_out[:], s_in[:])
                smod = cpool.tile([P, NI, 2], F32, tag="smod")  # 1 + s
                nc.scalar.activation(smod[:], sraw[:], AF.Copy, bias=1.0)
                tsq = cpool.tile([P, NI, 2], BF16, tag="tsq")   # (1 + s)^2
                nc.scalar.square(tsq[:], smod[:])

                # ---- demod d[o] = 1/sqrt(A_T.T @ tsq + eps) ----------------
                at_sb = ppool.tile([P, NI, COUT], BF16, tag="at_sb")
                nc.sync.dma_start(at_sb[:], at.rearrange("(t p) o -> p t o", p=P))
                dsb = cpool.tile([P, NO], F32, tag="dsb")
                dtmp = cpool.tile([P, NO], F32, tag="dtmp")
                epst = cpool.tile([P, 1], F32, tag="epst")
                nc.vector.memset(epst[:], EPS)
                for ot in range(NO):
                    o0 = ot * P
                    psd = psum_pool.tile([P, 2], F32, name="psd", tag="ps")
                    for it in range(NI):
                        nc.tensor.matmul(
                            psd[:],
                            at_sb[:, it, o0 : o0 + P],
                            tsq[:, it, :],
                            start=(it == 0),
                            stop=(it == NI - 1),
                        )
                    nc.scalar.activation(
                        dtmp[:, ot : ot + 1], psd[:, 0:1], AF.Sqrt, bias=epst[:]
                    )
                nc.vector.reciprocal(dsb[:], dtmp[:])

                # ---- modulated y-transformed image V[a] --------------------
                # B^T rows (points 0,1,-1,2,-2,inf):
                #   r0 = 4(d0-d2) - (d2-d4)      r3 =  2(d3-d1) + (d4-d2)
                #   r1 = -4(d1+d2) + (d3+d4)     r4 = -2(d3-d1) + (d4-d2)
                #   r2 = 4(d1-d2) - (d3-d4)      r5 = -4(d3-d1) - (d3-d5)
                vsb = []
                for it in range(NI):
                    t = ppool.tile([P, NA, TY, WP], BF16, tag=f"vsb{it}")
                    vsb.append(t)
                for it in range(NI):
                    ir = imgst.tile([P, HP, WP], BF16, name="ir", tag="ir")
                    nc.sync.dma_start(ir[:], img[it])
                    # modulation scale on ACT -- keeps the DVE stage1 chain
                    # (the iteration's critical path) as short as possible
                    nc.scalar.activation(
                        ir[:], ir[:], AF.Copy, scale=smod[:, it, 0:1]
                    )
                    dd = [ir[:, k : k + 4 * (TY - 1) + 1 : 4, :] for k in range(NA)]
                    v = vsb[it]

                    def w1(nm):
                        return w1pool.tile([P, TY, WP], BF16, name=nm, tag="w1")

                    # emit planes in A_ORDER consumption order (a=1 first);
                    # note h = d4 - d2 = -(d2 - d4) = -v_, so r3/r4 reuse v_.
                    p_ = w1("p_")
                    nc.vector.tensor_tensor(p_[:], dd[1], dd[2], OP.add)
                    q_ = w1("q_")
                    nc.vector.tensor_tensor(q_[:], dd[3], dd[4], OP.add)
                    nc.vector.scalar_tensor_tensor(
                        v[:, 1], p_[:], -4.0, q_[:], OP.mult, OP.add
                    )
                    e_ = w1("e_")
                    nc.vector.tensor_tensor(e_[:], dd[1], dd[2], OP.subtract)
                    f_ = w1("f_")
                    nc.vector.tensor_tensor(f_[:], dd[3], dd[4], OP.subtract)
                    nc.vector.scalar_tensor_tensor(
                        v[:, 2], e_[:], 4.0, f_[:], OP.mult, OP.subtract
                    )
                    g_ = w1("g_")
                    nc.vector.tensor_tensor(g_[:], dd[3], dd[1], OP.subtract)
                    v_ = w1("v_")
                    nc.vector.tensor_tensor(v_[:], dd[2], dd[4], OP.subtract)
                    nc.vector.scalar_tensor_tensor(
                        v[:, 3], g_[:], 2.0, v_[:], OP.mult, OP.subtract
                    )
                    nc.vector.scalar_tensor_tensor(
                        v[:, 4], g_[:], -2.0, v_[:], OP.mult, OP.subtract
                    )
                    u_ = w1("u_")
                    nc.vector.tensor_tensor(u_[:], dd[0], dd[2], OP.subtract)
                    nc.vector.scalar_tensor_tensor(
                        v[:, 0], u_[:], 4.0, v_[:], OP.mult, OP.subtract
                    )
                    z_ = w1("z_")
                    nc.vector.tensor_tensor(z_[:], dd[3], dd[5], OP.subtract)
                    nc.vector.scalar_tensor_tensor(
                        v[:, 5], g_[:], -4.0, z_[:], OP.mult, OP.subtract
                    )

                # ---- transformed weights U ---------------------------------
                usb = []
                for it in range(NI):
                    t = ppool.tile([P, NK, COUT], BF16, tag=f"usb{it}")
                    usb.append(t)
                    src = ut[:, it * P : (it + 1) * P, :]
                    for a in range(NA):
                        nc.sync.dma_start(
                            t[:, a * KS : (a + 1) * KS, :],
                            src[a * KS : (a + 1) * KS].rearrange("k p o -> p k o"),
                        )

                # ---- winograd-domain conv + inverse transform --------------
                # emit a-groups in consumption order so PSUM banks free early
                A_ORDER = (1, 2, 3, 4, 0, 5)
                for ot in range(NO):
                    o0 = ot * P
                    for tch in range(TCH):
                        ty0 = tch * TROWS
                        ps = {}
                        for a in A_ORDER:
                            p_ps = psum_pool.tile(
                                [P, NPIX], F32, name=f"ps{a}", tag="ps"
                            )
                            ps[a] = p_ps
                            j = 0
                            for it in range(NI):
                                for kx in range(KS):
                                    nc.tensor.matmul(
                                        p_ps[:],
                                        usb[it][:, a * KS + kx, o0 : o0 + P],
                                        vsb[it][:, a, ty0 : ty0 + TROWS, kx : kx + W],
                                        start=(j == 0),
                                        stop=(j == NK - 1),
                                    )
                                    j += 1

                        def iv(nm):
                            return invpool.tile([P, NPIX], F32, name=nm, tag="iv")

                        # inverse transform in fp32 from PSUM. DVE may read
                        # only ONE PSUM operand per op, so stage m1/m3 to
                        # SBUF via ACT copies first.
                        e1 = iv("e1")
                        nc.scalar.activation(e1[:], ps[1][:], AF.Copy)
                        e3 = iv("e3")
                        nc.scalar.activation(e3[:], ps[3][:], AF.Copy)
                        a1 = iv("a1")
                        nc.vector.tensor_tensor(a1[:], e1[:], ps[2][:], OP.subtract)
                        c1 = iv("c1")
                        nc.vector.tensor_tensor(c1[:], e1[:], ps[2][:], OP.add)
                        b1 = iv("b1")
                        nc.vector.tensor_tensor(b1[:], e3[:], ps[4][:], OP.subtract)
                        d1 = iv("d1")
                        nc.vector.tensor_tensor(d1[:], e3[:], ps[4][:], OP.add)
                        t_ = iv("t_")
                        nc.vector.tensor_tensor(t_[:], ps[0][:], c1[:], OP.add)
                        tt = iv("tt")
                        nc.vector.tensor_tensor(tt[:], t_[:], d1[:], OP.add)
                        y1r = iv("y1r")
                        nc.vector.scalar_tensor_tensor(
                            y1r[:], b1[:], 2.0, a1[:], OP.mult, OP.add
                        )
                        y2r = iv("y2r")
                        nc.vector.scalar_tensor_tensor(
                            y2r[:], d1[:], 4.0, c1[:], OP.mult, OP.add
                        )
                        y3r = iv("y3r")
                        nc.vector.scalar_tensor_tensor(
                            y3r[:], b1[:], 8.0, a1[:], OP.mult, OP.add
                        )
                        y3 = iv("y3")
                        nc.vector.tensor_tensor(y3[:], y3r[:], ps[5][:], OP.add)

                        # final demod scale + pack [ty, dy, x] (ACT)
                        y = ypool.tile([P, TROWS, MO, W], F32, name="y", tag="y")
                        for dy, src_t in enumerate((tt, y1r, y2r, y3)):
                            nc.scalar.activation(
                                y[:, :, dy, :],
                                src_t[:].rearrange("p (t x) -> p t x", x=W),
                                AF.Copy,
                                scale=dsb[:, ot : ot + 1],
                            )
                        nc.sync.dma_start(
                            out[o0 : o0 + P, tch * MO * TROWS : (tch + 1) * MO * TROWS, :],
                            y[:].rearrange("p t d x -> p (t d) x"),
                        )
    nc.compile()
    return nc


_NC_CACHE = None


def _get_nc():
    global _NC_CACHE
    if _NC_CACHE is None:
        _NC_CACHE = build_nc()
    return _NC_CACHE


def make_in_maps(img, s, weight):
    """Host-side input prep: shard over batch, static weight transforms."""
    img = np.asarray(img, dtype=np.float32)
    s = np.ascontiguousarray(np.asarray(s, dtype=np.float32))
    weight = np.asarray(weight, dtype=np.float32)
    bf = ml_dtypes.bfloat16
    imgp = np.zeros((img.shape[0], NI, P, HP, WP), dtype=bf)
    imgp[:, :, :, 1 : H + 1, 1 : W + 1] = img.reshape(-1, NI, P, H, W)
    # F(4,3) G (points 0,1,-1,2,-2,inf)
    G = np.array(
        [
            [1 / 4, 0, 0],
            [-1 / 6, -1 / 6, -1 / 6],
            [-1 / 6, 1 / 6, -1 / 6],
            [1 / 24, 1 / 12, 1 / 6],
            [1 / 24, -1 / 12, 1 / 6],
            [0, 0, 1],
        ],
        np.float64,
    )
    utv = np.einsum("ag,oigx->axio", G, weight.astype(np.float64))
    utv = np.ascontiguousarray(utv.reshape(NK, CIN, COUT)).astype(bf)
    atv = np.ascontiguousarray(
        (weight.astype(np.float64) ** 2).sum(axis=(2, 3)).T
    ).astype(bf)
    return [
        {"img": imgp[b], "s": s[b], "ut": utv, "at": atv} for b in range(N_CORES)
    ]


def kernel(img, s, weight):
    nc = _get_nc()
    in_maps = make_in_maps(img, s, weight)
    res = run_bass_kernel_spmd(nc, in_maps, list(range(N_CORES)))
    return np.stack([res.results[b]["out"] for b in range(N_CORES)], axis=0)
